# revision 37
# baseline (speedup 1.0000x reference)
"""CycleRNN (2-layer LSTM with output feedback) Trainium2 kernel.

Strategy: data-parallel over batch (B=256 -> 32 per core, 8 cores, zero
cross-core communication; the time loop is sequential per core).

Per-core design:
  * All weights live in SBUF in bf16, pre-transposed on the host so every
    DMA is contiguous.  Matmuls keep the WEIGHTS stationary (lhsT) and
    stream the batch (N=32) as the moving operand; with bf16 weights the
    FWL fast-weight-load path gives ~53ns per 128x128 weight tile.
  * All activations live transposed+packed: a [B=32, F] tensor is stored
    as [128 partitions = F mod 128, (F//128)*32 cols], so matmul outputs
    ([gate-tile, batch] in PSUM) feed the next matmul directly with no
    transposes anywhere in the time loop.
  * Cell state c stays fp32; h / z / y feed matmuls so they are produced
    directly in bf16 by the vector/scalar engines.
  * x is preloaded to SBUF (bf16, transposed on host) for all 300 steps;
    y_t is DMAd out per step.
  * The PE instruction stream is software-pipelined: the w_hh matmuls of
    layer l (which only need state from the previous step) fill the PE
    while the other engines run the LSTM cell math.
"""

import os
import sys
from contextlib import ExitStack

os.environ.setdefault("MYCRO_LOCAL_CACHE", "1")
sys.path.insert(0, "/opt/trn_rl_repo")

import numpy as np
import ml_dtypes

import concourse.bass as bass
import concourse.mybir as mybir
import concourse.tile as tile
from concourse.bass import ds
from concourse.bass_utils import run_bass_kernel_spmd

BF16 = ml_dtypes.bfloat16

T, B, IN, H, OUT, L = 300, 256, 512, 512, 128, 2
NCORES = 8
BC = B // NCORES          # 32 batch rows per core
G = 4 * H                 # 2048 gates per layer
KT = IN // 128            # 4 k-tiles per 512-feature dim
GM = G // 128             # 16 gate m-tiles
U = 2                     # steps unrolled per For_i iteration
STAGGERED = os.environ.get("KSTAG", "1") == "1"

f32 = mybir.dt.float32
bf16 = mybir.dt.bfloat16
AF = mybir.ActivationFunctionType


def build_program(zero_bias: bool, n_steps: int = T, reps: int = 1):
    nc = bass.Bass()

    # ---------------- DRAM parameters (host-packed layouts) ----------------
    xT_d = nc.declare_dram_parameter("xT", [n_steps, IN, BC], bf16, isOutput=False)
    w1_d = nc.declare_dram_parameter("w1T", [IN, H], bf16, isOutput=False)
    wih_d = nc.declare_dram_parameter("wihT", [L, H, G], bf16, isOutput=False)
    whh_d = nc.declare_dram_parameter("whhT", [L, H, G], bf16, isOutput=False)
    w2_d = nc.declare_dram_parameter("w2T", [H, OUT], bf16, isOutput=False)
    b1_d = nc.declare_dram_parameter("b1p", [128, KT], f32, isOutput=False)
    bs_d = nc.declare_dram_parameter("bsp", [L, 128, GM], f32, isOutput=False)
    b2_d = nc.declare_dram_parameter("b2p", [128, 1], f32, isOutput=False)
    ys_d = nc.declare_dram_parameter("ysT", [n_steps, OUT, BC], f32, isOutput=True)
    ys_flat = ys_d[:].rearrange("t o b -> (t o) b")
    xT_flat = xT_d[:].rearrange("t p b -> (t p) b")

    est = ExitStack()
    with est:
        # ---------------- persistent SBUF tensors ----------------
        w1_sb = est.enter_context(nc.sbuf_tensor([128, KT * H], bf16))
        wih_sb = est.enter_context(nc.sbuf_tensor([128, L * KT * G], bf16))
        whh_sb = est.enter_context(nc.sbuf_tensor([128, L * KT * G], bf16))
        w2_sb = est.enter_context(nc.sbuf_tensor([128, KT * OUT], bf16))
        b1_sb = est.enter_context(nc.sbuf_tensor([128, KT], f32))
        bs_sb = est.enter_context(nc.sbuf_tensor([128, L * GM], f32))
        b2_sb = est.enter_context(nc.sbuf_tensor([128, 1], f32))
        ybf_sb = est.enter_context(nc.sbuf_tensor([128, BC], bf16))
        # state ping-pong: index = step % 2
        h_sb = [[est.enter_context(nc.sbuf_tensor(f"h{l}_{s}", [128, 128], bf16))
                 for s in range(2)] for l in range(L)]
        c_sb = [[est.enter_context(nc.sbuf_tensor(f"c{l}_{s}", [128, 128], f32))
                 for s in range(2)] for l in range(L)]
        # PSUM: 8 banks, managed manually (full-bank tensors)
        zp = [est.enter_context(nc.psum_tensor(f"zp{s}", [128, 512], f32))
              for s in range(2)]
        g0p = [est.enter_context(nc.psum_tensor(f"g0p{s}", [128, 512], f32))
               for s in range(2)]
        g1p = [est.enter_context(nc.psum_tensor(f"g1p{s}", [128, 512], f32))
               for s in range(2)]
        yp = [est.enter_context(nc.psum_tensor(f"yp{s}", [128, 512], f32))
              for s in range(2)]

        with tile.TileContext(nc) as tc, \
                tc.tile_pool(name="work", bufs=2) as sb_pool:

            # ---------------- one-time loads (one DMA per tensor) ----------
            nc.sync.dma_start(
                out=w1_sb[:].rearrange("p (k h) -> p k h", k=KT),
                in_=w1_d[:].rearrange("(k p) h -> p k h", p=128),
            )
            nc.sync.dma_start(
                out=w2_sb[:].rearrange("p (k o) -> p k o", k=KT),
                in_=w2_d[:].rearrange("(k p) o -> p k o", p=128),
            )
            nc.sync.dma_start(
                out=wih_sb[:].rearrange("p (q g) -> p q g", g=G),
                in_=wih_d[:].rearrange("l h g -> (l h) g").rearrange(
                    "(q p) g -> p q g", p=128),
            )
            nc.sync.dma_start(
                out=whh_sb[:].rearrange("p (q g) -> p q g", g=G),
                in_=whh_d[:].rearrange("l h g -> (l h) g").rearrange(
                    "(q p) g -> p q g", p=128),
            )
            nc.sync.dma_start(out=b1_sb[:], in_=b1_d[:])
            nc.sync.dma_start(
                out=bs_sb[:].rearrange("p (l m) -> p l m", l=L),
                in_=bs_d[:].rearrange("l p m -> p l m"),
            )
            nc.sync.dma_start(out=b2_sb[:], in_=b2_d[:])
            # feedback buffer starts as x[0,:, -OUT:]  (t=0 uses ground truth)
            nc.sync.dma_start(out=ybf_sb[:], in_=xT_d[0, IN - OUT:IN, :])
            # Make SP observe every preamble DMA queue semaphore now, so the
            # loop back-edge drain's wait list only holds loop-body procs
            # (the ISA caps sync-wait commands per instruction).
            for tns in (w1_sb, w2_sb, wih_sb, whh_sb, b1_sb, bs_sb, b2_sb,
                        ybf_sb):
                n = 2 if tns.dtype == bf16 else 1
                nc.sync.value_load(tns[0:1, 0:n].bitcast(mybir.dt.int32))
            # initial h/c = 0 (slot 1 is read by step 0)
            for l in range(L):
                nc.vector.memset(h_sb[l][1][:], 0.0)
                nc.vector.memset(c_sb[l][1][:], 0.0)

            # w slice helpers: stationary lhsT tiles
            def w1_t(k, m):
                return w1_sb[:, k * H + m * 128: k * H + (m + 1) * 128]

            def wih_t(l, k, m):
                o = (l * KT + k) * G
                return wih_sb[:, o + m * 128: o + (m + 1) * 128]

            def whh_t(l, k, m):
                o = (l * KT + k) * G
                return whh_sb[:, o + m * 128: o + (m + 1) * 128]

            def w2_t(k):
                return w2_sb[:, k * OUT:(k + 1) * OUT]

            # ---------------- per-step emitters ----------------
            # start=True clears the has_written bits of the WHOLE psum bank,
            # so each bank epoch gets exactly one start (its very first MM)
            # and one stop (its last); later first-writes to other regions
            # overwrite-where-unwritten, which is the accumulation we want.
            def emit_gh(l, src_h, dst):
                """h-part of layer-l gates: dst[:, :512] (+)= w_hh[l] @ h."""
                for m in range(GM):
                    for k in range(KT):
                        nc.tensor.matmul(
                            dst[:, m * 32:(m + 1) * 32],
                            whh_t(l, k, m),
                            src_h[:, k * 32:(k + 1) * 32],
                            start=(m == 0 and k == 0),
                            stop=False,
                        )

            def emit_gin(l, src, dst):
                """input-part of layer-l gates (accumulates onto h-part)."""
                for m in range(GM):
                    for k in range(KT):
                        nc.tensor.matmul(
                            dst[:, m * 32:(m + 1) * 32],
                            wih_t(l, k, m),
                            src[:, k * 32:(k + 1) * 32],
                            start=False,
                            stop=(m == GM - 1 and k == KT - 1),
                        )

            def emit_xload(xrowbase):
                """x for U consecutive steps (transposed, bf16), one DMA.
                Loads all KT k-tiles; the fb k-tile is unused but keeping it
                makes the source contiguous (a single mergeable AP)."""
                x_t = sb_pool.tile([128, U * KT * BC], bf16, tag="x_t")
                nc.sync.dma_start(
                    out=x_t[:].rearrange("p (q b) -> p q b", b=BC),
                    in_=xT_flat[ds(xrowbase, U * IN), :].rearrange(
                        "(q p) b -> p q b", p=128),
                )
                return x_t

            def emit_z(x_t, s, z_dst):
                for m in range(KT):
                    for k in range(KT):
                        rhs = (ybf_sb[:] if k == KT - 1
                               else x_t[:, (s * KT + k) * BC:(s * KT + k + 1) * BC])
                        nc.tensor.matmul(
                            z_dst[:, m * 32:(m + 1) * 32],
                            w1_t(k, m),
                            rhs,
                            start=(m == 0 and k == 0),
                            stop=(m == KT - 1 and k == KT - 1),
                        )

            def emit_zact(z_src):
                z_bf = sb_pool.tile([128, 128], bf16, tag="z_bf")
                if zero_bias:
                    nc.scalar.activation(z_bf[:], z_src[:, 0:128], AF.Relu)
                else:
                    for m in range(KT):
                        nc.scalar.activation(
                            z_bf[:, m * 32:(m + 1) * 32],
                            z_src[:, m * 32:(m + 1) * 32],
                            AF.Relu,
                            bias=b1_sb[:, m:m + 1],
                        )
                return z_bf

            def emit_cell(l, s, g_src):
                """gates -> (h_new bf16, c_new f32) into slot s."""
                s_i = sb_pool.tile([128, 128], f32, tag="s_i")
                s_f = sb_pool.tile([128, 128], f32, tag="s_f")
                tg = sb_pool.tile([128, 128], f32, tag="tg")
                s_o = sb_pool.tile([128, 128], f32, tag="s_o")
                if zero_bias:
                    nc.scalar.activation(s_i[:], g_src[:, 0:128], AF.Sigmoid)
                    nc.scalar.activation(s_f[:], g_src[:, 128:256], AF.Sigmoid)
                    nc.scalar.activation(tg[:], g_src[:, 256:384], AF.Tanh)
                    nc.scalar.activation(s_o[:], g_src[:, 384:512], AF.Sigmoid)
                else:
                    outs = [s_i, s_i, s_i, s_i, s_f, s_f, s_f, s_f,
                            tg, tg, tg, tg, s_o, s_o, s_o, s_o]
                    funcs = [AF.Sigmoid] * 8 + [AF.Tanh] * 4 + [AF.Sigmoid] * 4
                    for m in range(GM):
                        nc.scalar.activation(
                            outs[m][:, (m % 4) * 32:(m % 4 + 1) * 32],
                            g_src[:, m * 32:(m + 1) * 32],
                            funcs[m],
                            bias=bs_sb[:, l * GM + m: l * GM + m + 1],
                        )
                c_old = c_sb[l][1 - s]
                c_new = c_sb[l][s]
                h_new = h_sb[l][s]
                t1 = sb_pool.tile([128, 128], f32, tag="t1")
                t2 = sb_pool.tile([128, 128], f32, tag="t2")
                tcn = sb_pool.tile([128, 128], f32, tag="tcn")
                nc.vector.tensor_mul(t1[:], s_f[:], c_old[:])
                nc.vector.tensor_mul(t2[:], s_i[:], tg[:])
                nc.vector.tensor_add(c_new[:], t1[:], t2[:])
                nc.scalar.activation(tcn[:], c_new[:], AF.Tanh)
                nc.vector.tensor_mul(h_new[:], s_o[:], tcn[:])
                return h_new

            def emit_y(src_h, dst):
                for k in range(KT):
                    nc.tensor.matmul(
                        dst[:, 0:BC],
                        w2_t(k),
                        src_h[:, k * 32:(k + 1) * 32],
                        start=(k == 0),
                        stop=(k == KT - 1),
                    )

            def emit_yout(y_src, y_pair, s):
                nc.scalar.activation(y_pair[:, s, :], y_src[:, 0:BC],
                                     AF.Identity, bias=b2_sb[:, 0:1])
                nc.scalar.activation(ybf_sb[:], y_src[:, 0:BC], AF.Identity,
                                     bias=b2_sb[:, 0:1])

            # prologue: h-part of layer-0 gates for step 0 (h=0, but also
            # initializes the PSUM accumulation group for the first GIN0)
            emit_gh(0, h_sb[0][1], g0p[0])

            n_iters = n_steps // U
            for _rep in range(reps):
              with tc.For_i(0, n_iters * U * 128, U * 128,
                            staggered_reset=STAGGERED) as it:
                x_t = emit_xload(it * 4)
                y_pair = sb_pool.tile([128, U, BC], f32, tag="y_pair")
                for s in range(U):
                    zt = zp[s % 2]
                    emit_z(x_t, s, zt)
                    emit_gh(1, h_sb[1][1 - s % 2], g1p[s % 2])
                    z_bf = emit_zact(zt)
                    emit_gin(0, z_bf, g0p[s % 2])
                    h0n = emit_cell(0, s % 2, g0p[s % 2])
                    emit_gin(1, h0n, g1p[s % 2])
                    # software pipeline: next step's layer-0 h-part
                    emit_gh(0, h0n, g0p[(s + 1) % 2])
                    h1n = emit_cell(1, s % 2, g1p[s % 2])
                    emit_y(h1n, yp[s % 2])
                    emit_yout(yp[s % 2], y_pair, s)
                nc.sync.dma_start(
                    out=ys_flat[ds(it, U * 128), :].rearrange(
                        "(t o) b -> o t b", t=U),
                    in_=y_pair[:],
                )

    _split_waits(nc)
    return nc


def _split_waits(nc, cap=1):
    """walrus encodes a single sync-wait command per instruction.  Hoist
    excess waits from any instruction onto inserted single-wait NOPs on
    the same engine — semantically identical, the engine just blocks on
    the NOPs first."""
    for bb in nc.m.functions[0].blocks:
        new_insts = []
        for inst in bb.instructions:
            if (inst.sync_info is not None
                    and len(inst.sync_info.on_wait or ()) > cap):
                waits = list(inst.sync_info.on_wait)
                head, tail = waits[:-cap], waits[-cap:]
                for w in head:
                    nop = mybir.InstNoOp(
                        name=nc.get_next_instruction_name(),
                        engine=inst.engine,
                        ins=[],
                        outs=[],
                        sync_info=mybir.SyncInfo(on_wait=[w], on_update=[]),
                    )
                    nc.register_instruction(nop)
                    new_insts.append(nop)
                inst.sync_info = mybir.SyncInfo(
                    on_wait=tail, on_update=inst.sync_info.on_update)
            new_insts.append(inst)
        bb.instructions[:] = new_insts


def _pack_inputs(x, w1, b1, w_ih, w_hh, b_ih, b_hh, w2, b2, n_steps=T):
    """Host-side packing shared by all cores (weights) + per-core x."""
    w1T = np.ascontiguousarray(w1.T).astype(BF16)
    wihT = np.ascontiguousarray(w_ih.transpose(0, 2, 1)).astype(BF16)
    whhT = np.ascontiguousarray(w_hh.transpose(0, 2, 1)).astype(BF16)
    w2T = np.ascontiguousarray(w2.T).astype(BF16)
    b1p = np.ascontiguousarray(b1.reshape(KT, 128).T).astype(np.float32)
    bsum = (b_ih + b_hh).astype(np.float32)
    bsp = np.ascontiguousarray(bsum.reshape(L, GM, 128).transpose(0, 2, 1))
    b2p = np.ascontiguousarray(b2.reshape(1, 128).T).astype(np.float32)
    shared = dict(w1T=w1T, wihT=wihT, whhT=whhT, w2T=w2T,
                  b1p=b1p, bsp=bsp, b2p=b2p)
    in_maps = []
    for c in range(NCORES):
        xs = x[:n_steps, c * BC:(c + 1) * BC, :]
        xT = np.ascontiguousarray(xs.transpose(0, 2, 1)).astype(BF16)
        in_maps.append(dict(xT=xT, **shared))
    zero_bias = (not b1.any()) and (not bsum.any()) and (not b2.any())
    return in_maps, zero_bias


def kernel(x, w1, b1, w_ih, w_hh, b_ih, b_hh, w2, b2):
    x = np.asarray(x, dtype=np.float32)
    args = [np.asarray(a, dtype=np.float32) for a in
            (w1, b1, w_ih, w_hh, b_ih, b_hh, w2, b2)]
    in_maps, zero_bias = _pack_inputs(x, *args)
    nc = build_program(zero_bias)
    res = run_bass_kernel_spmd(nc, in_maps, list(range(NCORES)))
    outs = [np.asarray(r["ysT"]).transpose(0, 2, 1) for r in res.results]
    return np.concatenate(outs, axis=1).astype(np.float32)


# revision 39
# speedup vs baseline: 1.0855x; 1.0855x over previous
"""CycleRNN (2-layer LSTM with output feedback) Trainium2 kernel.

Strategy: data-parallel over batch (B=256 -> 32 per core, 8 cores, zero
cross-core communication; the time loop is sequential per core).

Per-core design:
  * All weights live in SBUF in bf16, pre-transposed on the host so every
    DMA is contiguous.  Matmuls keep the WEIGHTS stationary (lhsT) and
    stream the batch (N=32) as the moving operand; with bf16 weights the
    FWL fast-weight-load path gives ~53ns per 128x128 weight tile.
  * All activations live transposed+packed: a [B=32, F] tensor is stored
    as [128 partitions = F mod 128, (F//128)*32 cols], so matmul outputs
    ([gate-tile, batch] in PSUM) feed the next matmul directly with no
    transposes anywhere in the time loop.
  * Cell state c stays fp32; h / z / y feed matmuls so they are produced
    directly in bf16 by the vector/scalar engines.
  * x is preloaded to SBUF (bf16, transposed on host) for all 300 steps;
    y_t is DMAd out per step.
  * The PE instruction stream is software-pipelined: the w_hh matmuls of
    layer l (which only need state from the previous step) fill the PE
    while the other engines run the LSTM cell math.
"""

import os
import sys
from contextlib import ExitStack

os.environ.setdefault("MYCRO_LOCAL_CACHE", "1")
sys.path.insert(0, "/opt/trn_rl_repo")

import numpy as np
import ml_dtypes

import concourse.bass as bass
import concourse.mybir as mybir
import concourse.tile as tile
from concourse.bass import ds
from concourse.bass_utils import run_bass_kernel_spmd

BF16 = ml_dtypes.bfloat16

T, B, IN, H, OUT, L = 300, 256, 512, 512, 128, 2
NCORES = 8
BC = B // NCORES          # 32 batch rows per core
G = 4 * H                 # 2048 gates per layer
KT = IN // 128            # 4 k-tiles per 512-feature dim
GM = G // 128             # 16 gate m-tiles
U = int(os.environ.get("KUNROLL", "2"))  # steps unrolled per For_i iteration
STAGGERED = os.environ.get("KSTAG", "0") == "1"
HINTS = os.environ.get("KHINT", "1") == "1"

f32 = mybir.dt.float32
bf16 = mybir.dt.bfloat16
AF = mybir.ActivationFunctionType


def build_program(zero_bias: bool, n_steps: int = T, reps: int = 1):
    nc = bass.Bass()

    # ---------------- DRAM parameters (host-packed layouts) ----------------
    xT_d = nc.declare_dram_parameter("xT", [n_steps, IN, BC], bf16, isOutput=False)
    w1_d = nc.declare_dram_parameter("w1T", [IN, H], bf16, isOutput=False)
    wih_d = nc.declare_dram_parameter("wihT", [L, H, G], bf16, isOutput=False)
    whh_d = nc.declare_dram_parameter("whhT", [L, H, G], bf16, isOutput=False)
    w2_d = nc.declare_dram_parameter("w2T", [H, OUT], bf16, isOutput=False)
    b1_d = nc.declare_dram_parameter("b1p", [128, KT], f32, isOutput=False)
    bs_d = nc.declare_dram_parameter("bsp", [L, 128, GM], f32, isOutput=False)
    b2_d = nc.declare_dram_parameter("b2p", [128, 1], f32, isOutput=False)
    ys_d = nc.declare_dram_parameter("ysT", [n_steps, OUT, BC], f32, isOutput=True)
    ys_flat = ys_d[:].rearrange("t o b -> (t o) b")
    xT_flat = xT_d[:].rearrange("t p b -> (t p) b")

    est = ExitStack()
    with est:
        # ---------------- persistent SBUF tensors ----------------
        w1_sb = est.enter_context(nc.sbuf_tensor([128, KT * H], bf16))
        wih_sb = est.enter_context(nc.sbuf_tensor([128, L * KT * G], bf16))
        whh_sb = est.enter_context(nc.sbuf_tensor([128, L * KT * G], bf16))
        w2_sb = est.enter_context(nc.sbuf_tensor([128, KT * OUT], bf16))
        b1_sb = est.enter_context(nc.sbuf_tensor([128, KT], f32))
        bs_sb = est.enter_context(nc.sbuf_tensor([128, L * GM], f32))
        b2_sb = est.enter_context(nc.sbuf_tensor([128, 1], f32))
        ybf_sb = est.enter_context(nc.sbuf_tensor([128, BC], bf16))
        # state ping-pong: index = step % 2
        h_sb = [[est.enter_context(nc.sbuf_tensor(f"h{l}_{s}", [128, 128], bf16))
                 for s in range(2)] for l in range(L)]
        c_sb = [[est.enter_context(nc.sbuf_tensor(f"c{l}_{s}", [128, 128], f32))
                 for s in range(2)] for l in range(L)]
        # PSUM: 8 banks, managed manually (full-bank tensors)
        zp = [est.enter_context(nc.psum_tensor(f"zp{s}", [128, 512], f32))
              for s in range(2)]
        g0p = [est.enter_context(nc.psum_tensor(f"g0p{s}", [128, 512], f32))
               for s in range(2)]
        g1p = [est.enter_context(nc.psum_tensor(f"g1p{s}", [128, 512], f32))
               for s in range(2)]
        yp = [est.enter_context(nc.psum_tensor(f"yp{s}", [128, 512], f32))
              for s in range(2)]

        with tile.TileContext(nc) as tc, \
                tc.tile_pool(name="work", bufs=2) as sb_pool:

            # ---------------- one-time loads (one DMA per tensor) ----------
            nc.sync.dma_start(
                out=w1_sb[:].rearrange("p (k h) -> p k h", k=KT),
                in_=w1_d[:].rearrange("(k p) h -> p k h", p=128),
            )
            nc.sync.dma_start(
                out=w2_sb[:].rearrange("p (k o) -> p k o", k=KT),
                in_=w2_d[:].rearrange("(k p) o -> p k o", p=128),
            )
            nc.sync.dma_start(
                out=wih_sb[:].rearrange("p (q g) -> p q g", g=G),
                in_=wih_d[:].rearrange("l h g -> (l h) g").rearrange(
                    "(q p) g -> p q g", p=128),
            )
            nc.sync.dma_start(
                out=whh_sb[:].rearrange("p (q g) -> p q g", g=G),
                in_=whh_d[:].rearrange("l h g -> (l h) g").rearrange(
                    "(q p) g -> p q g", p=128),
            )
            nc.sync.dma_start(out=b1_sb[:], in_=b1_d[:])
            nc.sync.dma_start(
                out=bs_sb[:].rearrange("p (l m) -> p l m", l=L),
                in_=bs_d[:].rearrange("l p m -> p l m"),
            )
            nc.sync.dma_start(out=b2_sb[:], in_=b2_d[:])
            # feedback buffer starts as x[0,:, -OUT:]  (t=0 uses ground truth)
            nc.sync.dma_start(out=ybf_sb[:], in_=xT_d[0, IN - OUT:IN, :])
            # Make SP observe every preamble DMA queue semaphore now, so the
            # loop back-edge drain's wait list only holds loop-body procs
            # (the ISA caps sync-wait commands per instruction).
            for tns in (w1_sb, w2_sb, wih_sb, whh_sb, b1_sb, bs_sb, b2_sb,
                        ybf_sb):
                n = 2 if tns.dtype == bf16 else 1
                nc.sync.value_load(tns[0:1, 0:n].bitcast(mybir.dt.int32))
            # initial h/c = 0 (slot 1 is read by step 0)
            for l in range(L):
                nc.vector.memset(h_sb[l][1][:], 0.0)
                nc.vector.memset(c_sb[l][1][:], 0.0)

            # w slice helpers: stationary lhsT tiles
            def w1_t(k, m):
                return w1_sb[:, k * H + m * 128: k * H + (m + 1) * 128]

            def wih_t(l, k, m):
                o = (l * KT + k) * G
                return wih_sb[:, o + m * 128: o + (m + 1) * 128]

            def whh_t(l, k, m):
                o = (l * KT + k) * G
                return whh_sb[:, o + m * 128: o + (m + 1) * 128]

            def w2_t(k):
                return w2_sb[:, k * OUT:(k + 1) * OUT]

            # ---------------- per-step emitters ----------------
            # start=True clears the has_written bits of the WHOLE psum bank,
            # so each bank epoch gets exactly one start (its very first MM)
            # and one stop (its last); later first-writes to other regions
            # overwrite-where-unwritten, which is the accumulation we want.
            def emit_gh(l, src_h, dst):
                """h-part of layer-l gates: dst[:, :512] (+)= w_hh[l] @ h."""
                for m in range(GM):
                    for k in range(KT):
                        nc.tensor.matmul(
                            dst[:, m * 32:(m + 1) * 32],
                            whh_t(l, k, m),
                            src_h[:, k * 32:(k + 1) * 32],
                            start=(m == 0 and k == 0),
                            stop=False,
                        )

            def emit_gin(l, src, dst):
                """input-part of layer-l gates (accumulates onto h-part)."""
                for m in range(GM):
                    for k in range(KT):
                        nc.tensor.matmul(
                            dst[:, m * 32:(m + 1) * 32],
                            wih_t(l, k, m),
                            src[:, k * 32:(k + 1) * 32],
                            start=False,
                            stop=(m == GM - 1 and k == KT - 1),
                        )

            def emit_xload(xrowbase):
                """x for U consecutive steps (transposed, bf16), one DMA.
                Loads all KT k-tiles; the fb k-tile is unused but keeping it
                makes the source contiguous (a single mergeable AP)."""
                x_t = sb_pool.tile([128, U * KT * BC], bf16, tag="x_t")
                nc.sync.dma_start(
                    out=x_t[:].rearrange("p (q b) -> p q b", b=BC),
                    in_=xT_flat[ds(xrowbase, U * IN), :].rearrange(
                        "(q p) b -> p q b", p=128),
                )
                return x_t

            def emit_z(x_t, s, z_dst):
                for m in range(KT):
                    for k in range(KT):
                        rhs = (ybf_sb[:] if k == KT - 1
                               else x_t[:, (s * KT + k) * BC:(s * KT + k + 1) * BC])
                        nc.tensor.matmul(
                            z_dst[:, m * 32:(m + 1) * 32],
                            w1_t(k, m),
                            rhs,
                            start=(m == 0 and k == 0),
                            stop=(m == KT - 1 and k == KT - 1),
                        )

            def emit_zact(z_src):
                z_bf = sb_pool.tile([128, 128], bf16, tag="z_bf")
                if zero_bias:
                    nc.scalar.activation(z_bf[:], z_src[:, 0:128], AF.Relu)
                else:
                    for m in range(KT):
                        nc.scalar.activation(
                            z_bf[:, m * 32:(m + 1) * 32],
                            z_src[:, m * 32:(m + 1) * 32],
                            AF.Relu,
                            bias=b1_sb[:, m:m + 1],
                        )
                return z_bf

            def emit_cell(l, s, g_src):
                """gates -> (h_new bf16, c_new f32) into slot s."""
                s_i = sb_pool.tile([128, 128], f32, tag="s_i")
                s_f = sb_pool.tile([128, 128], f32, tag="s_f")
                tg = sb_pool.tile([128, 128], f32, tag="tg")
                s_o = sb_pool.tile([128, 128], f32, tag="s_o")
                if zero_bias:
                    nc.scalar.activation(s_i[:], g_src[:, 0:128], AF.Sigmoid)
                    nc.scalar.activation(s_f[:], g_src[:, 128:256], AF.Sigmoid)
                    nc.scalar.activation(tg[:], g_src[:, 256:384], AF.Tanh)
                    nc.scalar.activation(s_o[:], g_src[:, 384:512], AF.Sigmoid)
                else:
                    outs = [s_i, s_i, s_i, s_i, s_f, s_f, s_f, s_f,
                            tg, tg, tg, tg, s_o, s_o, s_o, s_o]
                    funcs = [AF.Sigmoid] * 8 + [AF.Tanh] * 4 + [AF.Sigmoid] * 4
                    for m in range(GM):
                        nc.scalar.activation(
                            outs[m][:, (m % 4) * 32:(m % 4 + 1) * 32],
                            g_src[:, m * 32:(m + 1) * 32],
                            funcs[m],
                            bias=bs_sb[:, l * GM + m: l * GM + m + 1],
                        )
                c_old = c_sb[l][1 - s]
                c_new = c_sb[l][s]
                h_new = h_sb[l][s]
                t1 = sb_pool.tile([128, 128], f32, tag="t1")
                t2 = sb_pool.tile([128, 128], f32, tag="t2")
                tcn = sb_pool.tile([128, 128], f32, tag="tcn")
                nc.vector.tensor_mul(t1[:], s_f[:], c_old[:])
                nc.vector.tensor_mul(t2[:], s_i[:], tg[:])
                nc.vector.tensor_add(c_new[:], t1[:], t2[:])
                nc.scalar.activation(tcn[:], c_new[:], AF.Tanh)
                nc.vector.tensor_mul(h_new[:], s_o[:], tcn[:])
                return h_new

            def emit_y(src_h, dst):
                for k in range(KT):
                    nc.tensor.matmul(
                        dst[:, 0:BC],
                        w2_t(k),
                        src_h[:, k * 32:(k + 1) * 32],
                        start=(k == 0),
                        stop=(k == KT - 1),
                    )

            def emit_yout(y_src, y_pair, s):
                nc.scalar.activation(y_pair[:, s, :], y_src[:, 0:BC],
                                     AF.Identity, bias=b2_sb[:, 0:1])
                nc.scalar.activation(ybf_sb[:], y_src[:, 0:BC], AF.Identity,
                                     bias=b2_sb[:, 0:1])

            # prologue: h-part of layer-0 gates for step 0 (h=0, but also
            # initializes the PSUM accumulation group for the first GIN0)
            emit_gh(0, h_sb[0][1], g0p[0])

            n_iters = n_steps // U
            hint = (tuple(mybir.EngineType[e] for e in
                          ("PE", "Activation", "DVE", "SP"))
                    if HINTS else ())
            for _rep in range(reps):
              with tc.For_i(0, n_iters * U * 128, U * 128,
                            staggered_reset=STAGGERED,
                            hint_engines=hint) as it:
                x_t = emit_xload(it * 4)
                y_pair = sb_pool.tile([128, U, BC], f32, tag="y_pair")
                for s in range(U):
                    zt = zp[s % 2]
                    emit_z(x_t, s, zt)
                    emit_gh(1, h_sb[1][1 - s % 2], g1p[s % 2])
                    z_bf = emit_zact(zt)
                    emit_gin(0, z_bf, g0p[s % 2])
                    h0n = emit_cell(0, s % 2, g0p[s % 2])
                    emit_gin(1, h0n, g1p[s % 2])
                    # software pipeline: next step's layer-0 h-part
                    emit_gh(0, h0n, g0p[(s + 1) % 2])
                    h1n = emit_cell(1, s % 2, g1p[s % 2])
                    emit_y(h1n, yp[s % 2])
                    emit_yout(yp[s % 2], y_pair, s)
                nc.sync.dma_start(
                    out=ys_flat[ds(it, U * 128), :].rearrange(
                        "(t o) b -> o t b", t=U),
                    in_=y_pair[:],
                )

    _split_waits(nc)
    return nc


def _split_waits(nc, cap=1):
    """walrus encodes a single sync-wait command per instruction.  Hoist
    excess waits from any instruction onto inserted single-wait NOPs on
    the same engine — semantically identical, the engine just blocks on
    the NOPs first."""
    for bb in nc.m.functions[0].blocks:
        new_insts = []
        for inst in bb.instructions:
            if (inst.sync_info is not None
                    and len(inst.sync_info.on_wait or ()) > cap):
                waits = list(inst.sync_info.on_wait)
                head, tail = waits[:-cap], waits[-cap:]
                for w in head:
                    nop = mybir.InstNoOp(
                        name=nc.get_next_instruction_name(),
                        engine=inst.engine,
                        ins=[],
                        outs=[],
                        sync_info=mybir.SyncInfo(on_wait=[w], on_update=[]),
                    )
                    nc.register_instruction(nop)
                    new_insts.append(nop)
                inst.sync_info = mybir.SyncInfo(
                    on_wait=tail, on_update=inst.sync_info.on_update)
            new_insts.append(inst)
        bb.instructions[:] = new_insts


def _pack_inputs(x, w1, b1, w_ih, w_hh, b_ih, b_hh, w2, b2, n_steps=T):
    """Host-side packing shared by all cores (weights) + per-core x."""
    w1T = np.ascontiguousarray(w1.T).astype(BF16)
    wihT = np.ascontiguousarray(w_ih.transpose(0, 2, 1)).astype(BF16)
    whhT = np.ascontiguousarray(w_hh.transpose(0, 2, 1)).astype(BF16)
    w2T = np.ascontiguousarray(w2.T).astype(BF16)
    b1p = np.ascontiguousarray(b1.reshape(KT, 128).T).astype(np.float32)
    bsum = (b_ih + b_hh).astype(np.float32)
    bsp = np.ascontiguousarray(bsum.reshape(L, GM, 128).transpose(0, 2, 1))
    b2p = np.ascontiguousarray(b2.reshape(1, 128).T).astype(np.float32)
    shared = dict(w1T=w1T, wihT=wihT, whhT=whhT, w2T=w2T,
                  b1p=b1p, bsp=bsp, b2p=b2p)
    in_maps = []
    for c in range(NCORES):
        xs = x[:n_steps, c * BC:(c + 1) * BC, :]
        xT = np.ascontiguousarray(xs.transpose(0, 2, 1)).astype(BF16)
        in_maps.append(dict(xT=xT, **shared))
    zero_bias = (not b1.any()) and (not bsum.any()) and (not b2.any())
    return in_maps, zero_bias


def kernel(x, w1, b1, w_ih, w_hh, b_ih, b_hh, w2, b2):
    x = np.asarray(x, dtype=np.float32)
    args = [np.asarray(a, dtype=np.float32) for a in
            (w1, b1, w_ih, w_hh, b_ih, b_hh, w2, b2)]
    in_maps, zero_bias = _pack_inputs(x, *args)
    nc = build_program(zero_bias)
    res = run_bass_kernel_spmd(nc, in_maps, list(range(NCORES)))
    outs = [np.asarray(r["ysT"]).transpose(0, 2, 1) for r in res.results]
    return np.concatenate(outs, axis=1).astype(np.float32)
